# revision 17
# baseline (speedup 1.0000x reference)
"""Trainium2 Bass kernel for a causal attention block (RMSNorm + fused QKV +
RoPE + causal attention + output projection).

Full (unsharded) inputs in, full output out.  Internally shards across 8
NeuronCores: data-parallel over batch (2) x tensor-parallel over heads
(4 groups of 4 heads).  Each core computes a partial output-projection
(contraction over its 512 head-dims); the host sums the 4 partials per batch
and adds o_b.

Design (chunk-pipelined, bf16 matmuls with fp32 accumulation, SBUF-resident
k/v):
  - x ships pre-transposed [c, t] in bf16; weights/tables bf16.
  - Loop over 4 t-chunks of 512 tokens: RMS-norm scale -> QKV (per head) ->
    RoPE -> causal attention for the chunk (keys/values accumulated in SBUF,
    no DRAM round-trip) -> output projection; the next chunk's x DMA, norm
    reduction and weight loads are emitted mid-chunk so the PE never waits
    on HBM.
  - Attention interleaves two heads and pipelines exp two j-blocks deep so
    the softmax exp (scalar engine) hides under the other head's matmuls;
    the causal mask is pre-loaded into PSUM with a tiny matmul (start=True,
    stop=False) so the QK matmul accumulates onto it.
  - Softmax row-sums via an all-ones 128x128 stationary (result lands
    broadcast across partitions -> reciprocal_approx_fast, no gpsimd
    broadcast needed).
  - rotate_half runs as a PE matmul against a 128x128 swap-permutation
    matrix (DVE ops cannot cross partition bases); for chunks 1-3 the k
    rotate uses a latency-tolerant SBUF->SBUF DMA partition swap instead.
  - RMSNorm rsqrt = exp(-0.5*ln(m)): ln/exp/identity/square share one
    scalar-engine activation table, avoiding ACT_TABLE_LOAD swaps in the
    exp-heavy attention phase.
"""

import os
import sys

import numpy as np

for _p in ("/opt/trn_rl_repo", "/opt/pypackages"):
    if _p not in sys.path and os.path.isdir(_p):
        sys.path.append(_p)

import concourse.mybir as mybir
import concourse.tile as tile
from concourse import bacc
from concourse.bass_utils import run_bass_kernel_spmd

try:
    import ml_dtypes
    BF16NP = ml_dtypes.bfloat16
except Exception:  # pragma: no cover
    BF16NP = None

F32 = mybir.dt.float32
BF = mybir.dt.bfloat16
AF = mybir.ActivationFunctionType

B, T, C = 2, 2048, 2048
H, D = 16, 128
EPS = 1e-5
ROPE_BASE = 10000.0
NCORES = 8
HPC = 4            # heads per core
KC = C // 128      # 16 contraction blocks
NT = T // 512      # 4 t-chunks
SCALE = 1.0 / float(np.sqrt(D))
NEG = -1e30

last_exec_time_ns = None
_cache = {}


def _build_nc():
    nc = bacc.Bacc("TRN2", target_bir_lowering=False, debug=False,
                   num_devices=NCORES)
    x_d = nc.declare_dram_parameter("xT", [128, NT, KC, 512], BF,
                                    isOutput=False)
    w_d = nc.declare_dram_parameter("wqkv", [128, 3 * HPC, KC, 128], BF,
                                    isOutput=False)
    b_d = nc.declare_dram_parameter("bqkv", [128, 3 * HPC], F32,
                                    isOutput=False)
    ow_d = nc.declare_dram_parameter("owT", [128, HPC, C], BF, isOutput=False)
    cos_d = nc.declare_dram_parameter("cosT", [128, T], BF, isOutput=False)
    sin_d = nc.declare_dram_parameter("sinmT", [128, T], BF, isOutput=False)
    tri_d = nc.declare_dram_parameter("triT", [128, 128], BF, isOutput=False)
    id_d = nc.declare_dram_parameter("ident", [128, 128], BF, isOutput=False)
    on_d = nc.declare_dram_parameter("onesf", [128, 128], BF, isOutput=False)
    sw_d = nc.declare_dram_parameter("swapm", [128, 128], BF, isOutput=False)
    out_d = nc.declare_dram_parameter("out", [128, T // 128, C], BF,
                                      isOutput=True)

    with tile.TileContext(nc) as tc:
        with (tc.tile_pool(name="constp", bufs=1) as constp,
              tc.tile_pool(name="kvp", bufs=1) as kvp,
              tc.tile_pool(name="xkp", bufs=2) as xkp,
              tc.tile_pool(name="wp", bufs=3) as wp,
              tc.tile_pool(name="stg", bufs=2) as stg,
              tc.tile_pool(name="mmp", bufs=2, space="PSUM") as mmp,
              tc.tile_pool(name="scpp", bufs=2, space="PSUM") as scpp,
              tc.tile_pool(name="uop", bufs=2, space="PSUM") as uop,
              tc.tile_pool(name="rsp", bufs=2, space="PSUM") as rsp):

            # ---------------- constants / first prefetches ----------------
            xns = {}
            xns[0] = xkp.tile([128, KC, 512], BF, tag="xn", name="xn0")
            nc.sync.dma_start(out=xns[0][:, 0:2, :], in_=x_d[:, 0, 0:2, :])
            onesf = constp.tile([128, 128], BF, tag="onesf")
            nc.sync.dma_start(out=onesf, in_=on_d[:, :])
            for lo, hi in ((2, 6), (6, 11), (11, 16)):
                nc.sync.dma_start(out=xns[0][:, lo:hi, :],
                                  in_=x_d[:, 0, lo:hi, :])
            bqkv = constp.tile([128, 3 * HPC], F32, tag="bqkv")
            nc.sync.dma_start(out=bqkv, in_=b_d[:, :])
            wms = {}
            wms[(0, 0)] = wp.tile([128, 3, KC, 128], BF, tag="wmh", name="wmh00")
            for sec3 in range(3):
                nc.sync.dma_start(out=wms[(0, 0)][:, sec3],
                                  in_=w_d[:, sec3])
            cosT = constp.tile([128, T], BF, tag="cosT")
            nc.sync.dma_start(out=cosT, in_=cos_d[:, :])
            sinmT = constp.tile([128, T], BF, tag="sinmT")
            nc.sync.dma_start(out=sinmT, in_=sin_d[:, :])
            ident = constp.tile([128, 128], BF, tag="ident")
            nc.sync.dma_start(out=ident, in_=id_d[:, :])
            triT = constp.tile([128, 128], BF, tag="triT")
            nc.sync.dma_start(out=triT, in_=tri_d[:, :])
            swapm = constp.tile([128, 128], BF, tag="swapm")
            nc.sync.dma_start(out=swapm, in_=sw_d[:, :])
            owT = constp.tile([128, HPC, C], BF, tag="owT")

            epst = constp.tile([1, 1], F32, tag="epst")
            nc.vector.memset(epst, EPS)
            kTs = [kvp.tile([128, T], BF, tag=f"kT{h}", name=f"kT{h}")
                   for h in range(HPC)]
            vtrs = [kvp.tile([128, T // 128, 128], BF, tag=f"vtr{h}",
                             name=f"vtr{h}")
                    for h in range(HPC)]

            xsqs = {}

            def emit_xsq(n):
                """x^2 tiles for chunk n (split scalar/vector engines)."""
                xn = xns[n]
                tiles = []
                for kb in range(KC):
                    xsq = stg.tile([128, 512], BF, tag="xsq", bufs=8,
                                   name="xsq")
                    nc.vector.tensor_mul(xsq, xn[:, kb, :], xn[:, kb, :])
                    tiles.append(xsq)
                xsqs[n] = tiles

            def emit_norm_reduce(n):
                """RMS scale for chunk n: xn *= rsqrt(mean(x^2)+eps)."""
                xn = xns[n]
                ss = uop.tile([128, 512], F32, tag="uo")
                for kb in range(KC):
                    nc.tensor.matmul(ss, onesf, xsqs[n][kb],
                                     start=(kb == 0), stop=(kb == KC - 1))
                lnm = stg.tile([1, 512], F32, tag="lnm")
                nc.scalar.activation(out=lnm, in_=ss[0:1, :], func=AF.Ln,
                                     bias=epst, scale=float(1.0 / C))
                r1 = stg.tile([1, 512], BF, tag="r1")
                nc.scalar.activation(out=r1, in_=lnm, func=AF.Exp, scale=-0.5)
                rbT = stg.tile([128, 512], BF, tag="rbT")
                nc.gpsimd.partition_broadcast(rbT, r1)
                for kb in range(KC):
                    nc.vector.tensor_mul(xn[:, kb, :], xn[:, kb, :], rbT)

            def rope_swap(src):
                """rot_half permutation via PE: qsw = J @ src (PSUM f32)."""
                qsw = scpp.tile([128, 512], F32, tag="scp")
                nc.tensor.matmul(qsw, swapm, src, start=True, stop=True)
                return qsw

            def rope_finish(src, qsw, dest_ap, nsl):
                """dest = src*cos + qsw*sinm (sign folded in sinmT)."""
                qsin = stg.tile([128, 512], BF, tag="qsin")
                nc.vector.tensor_mul(qsin, qsw, sinmT[:, nsl])
                qcos = stg.tile([128, 512], BF, tag="qcos")
                nc.vector.tensor_mul(qcos, src, cosT[:, nsl])
                nc.vector.tensor_add(dest_ap, qcos, qsin)

            def emit_kvq(n, h, qTs):
                """QKV projection + RoPE/v-transpose for head h, chunk n.

                PE-stream order: q-MMs, v-MMs, Jswap(q), k-MMs, vtrans(v) --
                each dependent PE op hides behind the next 16-matmul chain.
                The k rotate goes through a DMA partition swap (except chunk
                0, whose k feeds attention immediately)."""
                nsl = slice(n * 512, (n + 1) * 512)
                xn = xns[n]
                wmh = wms[(n, h)]

                def proj(sec, ptag="mm"):
                    if ptag == "mm":
                        ps = mmp.tile([128, 512], F32, tag="mm", name="ps")
                    else:
                        ps = rsp.tile([128, 512], F32, tag="rs", name="ps2")
                    for kb in range(KC):
                        nc.tensor.matmul(ps, wmh[:, sec, kb, :],
                                         xn[:, kb, :],
                                         start=(kb == 0), stop=(kb == KC - 1))
                    qs = stg.tile([128, 512], BF, tag="qs", bufs=6,
                                  name="qs")
                    nc.scalar.activation(out=qs, in_=ps, func=AF.Identity,
                                         bias=bqkv[:, 3 * h + sec:
                                                   3 * h + sec + 1])
                    return qs

                qs_q = proj(2, ptag="rs" if h == 0 else "mm")
                qs_v = proj(1)
                qsw_q = rope_swap(qs_q)
                qs_k = proj(0)
                qT = stg.tile([128, 512], BF, tag=f"qT{h}")
                rope_finish(qs_q, qsw_q, qT, nsl)
                qTs[h] = qT
                vtp = scpp.tile([128, 4, 128], BF, tag="scp")
                for i2 in range(4):
                    nc.tensor.transpose(
                        vtp[:, i2, :], qs_v[:, i2 * 128:(i2 + 1) * 128],
                        ident)
                nc.vector.tensor_copy(vtrs[h][:, 4 * n:4 * n + 4, :], vtp)
                if n == 0:
                    qsw_k = rope_swap(qs_k)
                else:
                    qsw_k = stg.tile([128, 512], BF, tag="qswk", bufs=3)
                    nc.sync.dma_start(out=qsw_k[0:64, :],
                                      in_=qs_k[64:128, :])
                    nc.sync.dma_start(out=qsw_k[64:128, :],
                                      in_=qs_k[0:64, :])
                rope_finish(qs_k, qsw_k, kTs[h][:, nsl], nsl)

            def emit_attn_pair(n, pair, qTs, aTc):
                """Causal attention for two heads, j-loops interleaved."""
                nj = 4 * n + 4
                uo = {}
                rs = {}
                pts = {}
                usls = {}
                for h in pair:
                    uo[h] = uop.tile([128, 512], F32, tag="uo", name=f"uo{h}")
                    rs[h] = rsp.tile([128, 512], F32, tag="rs", name=f"rs{h}")

                def pv_rs(h, jb):
                    usl = usls[(h, jb)]
                    pt = pts[(h, jb)]
                    nc.tensor.matmul(uo[h][:, usl], vtrs[h][:, jb, :],
                                     pt[:, usl],
                                     start=(jb == 0), stop=(jb == nj - 1))
                    nc.tensor.matmul(rs[h][:, usl], onesf, pt[:, usl],
                                     start=(jb == 0), stop=(jb == nj - 1))

                for jb in range(nj):
                    r = jb - 4 * n
                    scps = {}
                    for h in pair:
                        jsl = slice(jb * 128, (jb + 1) * 128)
                        scp = scpp.tile([128, 512], F32, tag="scp")
                        if r >= 0:
                            u0 = 128 * r
                            usl = slice(u0, 512)
                            # causal mask pre-load: scp[:,u0:u0+128] = triT.T
                            nc.tensor.matmul(scp[:, u0:u0 + 128], triT,
                                             ident, start=True, stop=False)
                            nc.tensor.matmul(scp[:, usl], kTs[h][:, jsl],
                                             qTs[h][:, usl],
                                             start=False, stop=True)
                        else:
                            usl = slice(0, 512)
                            nc.tensor.matmul(scp, kTs[h][:, jsl], qTs[h],
                                             start=True, stop=True)
                        usls[(h, jb)] = usl
                        scps[h] = scp
                    if jb > 1:
                        for h in pair:
                            pv_rs(h, jb - 2)
                    for h in pair:
                        usl = usls[(h, jb)]
                        pt = stg.tile([128, 512], BF, tag="pt", bufs=8)
                        nc.scalar.activation(out=pt[:, usl],
                                             in_=scps[h][:, usl],
                                             func=AF.Exp, scale=SCALE)
                        pts[(h, jb)] = pt
                for jb2 in (nj - 2, nj - 1):
                    for h in pair:
                        pv_rs(h, jb2)
                for h in pair:
                    rcs = stg.tile([128, 512], F32, tag="rcs")
                    nc.vector.reciprocal_approx_fast(out=rcs, in_=rs[h])
                    nc.vector.tensor_mul(aTc[:, h, :], uo[h], rcs)

            def emit_oproj(n, aTc):
                for tb in range(4):
                    ost = stg.tile([128, C], BF, tag="ost", bufs=3)
                    for nn in range(4):
                        pso = mmp.tile([128, 512], F32, tag="mm")
                        for cb in range(HPC):
                            nc.tensor.matmul(
                                pso, aTc[:, cb, tb * 128:(tb + 1) * 128],
                                owT[:, cb, nn * 512:(nn + 1) * 512],
                                start=(cb == 0), stop=(cb == HPC - 1))
                        if nn % 2 == 0:
                            nc.scalar.activation(
                                out=ost[:, nn * 512:(nn + 1) * 512],
                                in_=pso, func=AF.Copy)
                        else:
                            nc.vector.tensor_copy(
                                ost[:, nn * 512:(nn + 1) * 512], pso)
                    nc.sync.dma_start(out=out_d[:, 4 * n + tb, :], in_=ost)

            # ---------------- main chunk pipeline --------------------------
            slots = [(n, h) for n in range(NT) for h in range(HPC)]

            def prefetch_w(idx):
                if idx >= len(slots) or slots[idx] in wms:
                    return
                n_, h_ = slots[idx]
                wms[(n_, h_)] = wp.tile([128, 3, KC, 128], BF,
                                        tag="wmh", name=f"wmh{n_}{h_}")
                nc.sync.dma_start(out=wms[(n_, h_)],
                                  in_=w_d[:, 3 * h_:3 * h_ + 3])

            prefetch_w(1)
            nc.sync.dma_start(out=owT, in_=ow_d[:, :, :])
            emit_xsq(0)
            emit_norm_reduce(0)
            for n in range(NT):
                if n < NT - 1:
                    xns[n + 1] = xkp.tile([128, KC, 512], BF, tag="xn",
                                          name=f"xn{n + 1}")
                    nc.sync.dma_start(out=xns[n + 1], in_=x_d[:, n + 1])
                qTs = {}
                aTc = stg.tile([128, HPC, 512], BF, tag="aTc")
                for h in (0, 1):
                    prefetch_w(4 * n + h + 2)
                    emit_kvq(n, h, qTs)
                emit_attn_pair(n, (0, 1), qTs, aTc)
                if n < NT - 1:
                    emit_xsq(n + 1)
                prefetch_w(4 * n + 4)
                emit_kvq(n, 2, qTs)
                if n < NT - 1:
                    emit_norm_reduce(n + 1)
                prefetch_w(4 * n + 5)
                emit_kvq(n, 3, qTs)
                emit_attn_pair(n, (2, 3), qTs, aTc)
                emit_oproj(n, aTc)

    nc.compile()
    return nc


def _get_nc():
    if "nc" not in _cache:
        _cache["nc"] = _build_nc()
    return _cache["nc"]


def _host_prep(x, rms_weight, qkv_w, qkv_b, o_w):
    """Build the per-core input maps (all bf16 except biases/masks)."""
    x = np.asarray(x, dtype=np.float32)
    rms_weight = np.asarray(rms_weight, dtype=np.float32)
    qkv_w = np.asarray(qkv_w, dtype=np.float32)
    qkv_b = np.asarray(qkv_b, dtype=np.float32)
    o_w = np.asarray(o_w, dtype=np.float32)

    w_eff = qkv_w * rms_weight[None, :]

    pos = np.arange(T, dtype=np.float32)
    inv_freq = (1.0 / (ROPE_BASE ** (np.arange(0, D, 2, dtype=np.float32)
                                     / D))).astype(np.float32)
    Fr = pos[None, :] * inv_freq[:, None]           # [64, T]
    cos_h = np.cos(Fr)
    sin_h = np.sin(Fr)
    cosT = np.concatenate([cos_h, cos_h], axis=0).astype(BF16NP)   # [128, T]
    sinmT = np.concatenate([-sin_h, sin_h], axis=0).astype(BF16NP)

    iu = np.arange(128)
    # mask[k, q] = 0 if q >= k else NEG ; triT = mask.T (lhsT for PE preload)
    mask = np.where(iu[None, :] >= iu[:, None], 0.0, NEG).astype(np.float32)
    triT = np.ascontiguousarray(mask.T).astype(BF16NP)
    identb = np.eye(128, dtype=np.float32).astype(BF16NP)
    swapm = np.zeros((128, 128), dtype=np.float32)
    swapm[np.arange(64), np.arange(64) + 64] = 1.0   # J[i, i+64] = 1
    swapm[np.arange(64) + 64, np.arange(64)] = 1.0   # J[i+64, i] = 1
    swapm = swapm.astype(BF16NP)
    onesb = np.ones((128, 128), dtype=np.float32).astype(BF16NP)

    # section order per head: k, v, q
    secoff = (C, 2 * C, 0)

    in_maps = []
    for core in range(NCORES):
        b = core // 4
        g = core % 4
        # xT packed [p, n, kb, ts]
        xT = np.ascontiguousarray(
            x[b].T.reshape(KC, 128, NT, 512).transpose(1, 2, 0, 3)
        ).astype(BF16NP)
        wblocks = []
        bcols = []
        for h in range(HPC):
            head = 4 * g + h
            for sec in range(3):
                r0 = secoff[sec] + head * 128
                sub = w_eff[r0:r0 + 128, :]                 # [128m, C]
                wblocks.append(sub.T.reshape(KC, 128, 128).transpose(1, 0, 2))
                bcols.append(qkv_b[r0:r0 + 128])
        wqkv = np.ascontiguousarray(
            np.stack(wblocks, axis=1)).astype(BF16NP)       # [128,12,KC,128]
        bq = np.ascontiguousarray(np.stack(bcols, axis=1)).astype(np.float32)
        owT = np.ascontiguousarray(
            o_w[:, g * 512:(g + 1) * 512].reshape(C, HPC, 128)
            .transpose(2, 1, 0)).astype(BF16NP)             # [128, HPC, C]
        in_maps.append({
            "xT": xT,
            "wqkv": wqkv,
            "bqkv": bq,
            "owT": owT,
            "cosT": cosT,
            "sinmT": sinmT,
            "triT": triT,
            "ident": identb,
            "swapm": swapm,
            "onesf": onesb,
        })
    return in_maps


def kernel(x, rms_weight, qkv_w, qkv_b, o_w, o_b):
    global last_exec_time_ns
    o_b = np.asarray(o_b, dtype=np.float32)
    in_maps = _host_prep(x, rms_weight, qkv_w, qkv_b, o_w)
    nc = _get_nc()

    trace = bool(int(os.environ.get("BASSK_TRACE", "0")))
    if trace:
        try:
            import ntff_shim
            ntff_shim.install()
        except Exception:
            pass
    res = None
    for attempt in range(4):
        try:
            res = run_bass_kernel_spmd(nc, in_maps, list(range(NCORES)),
                                       trace=trace)
            break
        except ImportError:
            # trace plumbing unavailable in this environment
            trace = False
        except Exception:
            if attempt == 3:
                raise
            import time
            time.sleep(5)
    last_exec_time_ns = res.exec_time_ns

    out = np.empty((B, T, C), dtype=np.float32)
    for b in range(B):
        acc = None
        for g in range(4):
            part = np.asarray(res.results[4 * b + g]["out"],
                              dtype=np.float32)
            part = part.transpose(1, 0, 2).reshape(T, C)
            acc = part if acc is None else acc + part
        out[b] = acc + o_b[None, :]
    return out


# revision 18
# speedup vs baseline: 1.1847x; 1.1847x over previous
"""Trainium2 Bass kernel for a causal attention block (RMSNorm + fused QKV +
RoPE + causal attention + output projection).

Full (unsharded) inputs in, full output out.  Internally shards across 8
NeuronCores: data-parallel over batch (2) x tensor-parallel over heads
(4 groups of 4 heads).  Each core computes a partial output-projection
(contraction over its 512 head-dims); the host sums the 4 partials per batch
and adds o_b.

Design (chunk-pipelined, bf16 matmuls with fp32 accumulation, SBUF-resident
k/v):
  - x ships pre-transposed [c, t] in bf16; weights/tables bf16.
  - Loop over 4 t-chunks of 512 tokens: RMS-norm scale -> QKV (per head) ->
    RoPE -> causal attention for the chunk (keys/values accumulated in SBUF,
    no DRAM round-trip) -> output projection; the next chunk's x DMA, norm
    reduction and weight loads are emitted mid-chunk so the PE never waits
    on HBM.
  - Attention interleaves two heads and pipelines exp two j-blocks deep so
    the softmax exp (scalar engine) hides under the other head's matmuls;
    the causal mask is pre-loaded into PSUM with a tiny matmul (start=True,
    stop=False) so the QK matmul accumulates onto it.
  - Softmax row-sums via an all-ones 128x128 stationary (result lands
    broadcast across partitions -> reciprocal_approx_fast, no gpsimd
    broadcast needed).
  - rotate_half runs as a PE matmul against a 128x128 swap-permutation
    matrix (DVE ops cannot cross partition bases); for chunks 1-3 the k
    rotate uses a latency-tolerant SBUF->SBUF DMA partition swap instead.
  - RMSNorm rsqrt = exp(-0.5*ln(m)): ln/exp/identity/square share one
    scalar-engine activation table, avoiding ACT_TABLE_LOAD swaps in the
    exp-heavy attention phase.
"""

import os
import sys

import numpy as np

for _p in ("/opt/trn_rl_repo", "/opt/pypackages"):
    if _p not in sys.path and os.path.isdir(_p):
        sys.path.append(_p)

import concourse.mybir as mybir
import concourse.tile as tile
from concourse import bacc
from concourse.bass_utils import run_bass_kernel_spmd

try:
    import ml_dtypes
    BF16NP = ml_dtypes.bfloat16
except Exception:  # pragma: no cover
    BF16NP = None

F32 = mybir.dt.float32
BF = mybir.dt.bfloat16
AF = mybir.ActivationFunctionType

B, T, C = 2, 2048, 2048
H, D = 16, 128
EPS = 1e-5
ROPE_BASE = 10000.0
NCORES = 8
HPC = 4            # heads per core
KC = C // 128      # 16 contraction blocks
NT = T // 512      # 4 t-chunks
SCALE = 1.0 / float(np.sqrt(D))
NEG = -1e30

last_exec_time_ns = None
_cache = {}


def _build_nc():
    nc = bacc.Bacc("TRN2", target_bir_lowering=False, debug=False,
                   num_devices=NCORES)
    x_d = nc.declare_dram_parameter("xT", [128, NT, KC, 512], BF,
                                    isOutput=False)
    w_d = nc.declare_dram_parameter("wqkv", [128, 3 * HPC, KC, 128], BF,
                                    isOutput=False)
    b_d = nc.declare_dram_parameter("bqkv", [128, 3 * HPC], F32,
                                    isOutput=False)
    ow_d = nc.declare_dram_parameter("owT", [128, HPC, C], BF, isOutput=False)
    cos_d = nc.declare_dram_parameter("cosT", [128, T], BF, isOutput=False)
    sin_d = nc.declare_dram_parameter("sinmT", [128, T], BF, isOutput=False)
    tri_d = nc.declare_dram_parameter("triT", [128, 128], BF, isOutput=False)
    id_d = nc.declare_dram_parameter("ident", [128, 128], BF, isOutput=False)
    on_d = nc.declare_dram_parameter("onesf", [128, 128], BF, isOutput=False)
    sw_d = nc.declare_dram_parameter("swapm", [128, 128], BF, isOutput=False)
    out_d = nc.declare_dram_parameter("out", [128, T // 128, C], BF,
                                      isOutput=True)

    with tile.TileContext(nc) as tc:
        with (tc.tile_pool(name="constp", bufs=1) as constp,
              tc.tile_pool(name="kvp", bufs=1) as kvp,
              tc.tile_pool(name="xkp", bufs=2) as xkp,
              tc.tile_pool(name="wp", bufs=3) as wp,
              tc.tile_pool(name="stg", bufs=2) as stg,
              tc.tile_pool(name="mmp", bufs=2, space="PSUM") as mmp,
              tc.tile_pool(name="scpp", bufs=2, space="PSUM") as scpp,
              tc.tile_pool(name="uop", bufs=2, space="PSUM") as uop,
              tc.tile_pool(name="rsp", bufs=2, space="PSUM") as rsp):

            # ---------------- constants / first prefetches ----------------
            xns = {}
            xns[0] = xkp.tile([128, KC, 512], BF, tag="xn", name="xn0")
            nc.sync.dma_start(out=xns[0][:, 0:2, :], in_=x_d[:, 0, 0:2, :])
            onesf = constp.tile([128, 128], BF, tag="onesf")
            nc.sync.dma_start(out=onesf, in_=on_d[:, :])
            for lo, hi in ((2, 6), (6, 11), (11, 16)):
                nc.sync.dma_start(out=xns[0][:, lo:hi, :],
                                  in_=x_d[:, 0, lo:hi, :])
            bqkv = constp.tile([128, 3 * HPC], F32, tag="bqkv")
            nc.sync.dma_start(out=bqkv, in_=b_d[:, :])
            wms = {}
            wms[(0, 0)] = wp.tile([128, 3, KC, 128], BF, tag="wmh", name="wmh00")
            for sec3 in range(3):
                nc.sync.dma_start(out=wms[(0, 0)][:, sec3],
                                  in_=w_d[:, sec3])
            cosT = constp.tile([128, T], BF, tag="cosT")
            nc.sync.dma_start(out=cosT, in_=cos_d[:, :])
            sinmT = constp.tile([128, T], BF, tag="sinmT")
            nc.sync.dma_start(out=sinmT, in_=sin_d[:, :])
            ident = constp.tile([128, 128], BF, tag="ident")
            nc.sync.dma_start(out=ident, in_=id_d[:, :])
            triT = constp.tile([128, 128], BF, tag="triT")
            nc.sync.dma_start(out=triT, in_=tri_d[:, :])
            swapm = constp.tile([128, 128], BF, tag="swapm")
            nc.sync.dma_start(out=swapm, in_=sw_d[:, :])
            owT = constp.tile([128, HPC, C], BF, tag="owT")

            epst = constp.tile([1, 1], F32, tag="epst")
            nc.vector.memset(epst, EPS)
            kTs = [kvp.tile([128, T], BF, tag=f"kT{h}", name=f"kT{h}")
                   for h in range(HPC)]
            vtrs = [kvp.tile([128, T // 128, 128], BF, tag=f"vtr{h}",
                             name=f"vtr{h}")
                    for h in range(HPC)]

            xsqs = {}

            def emit_xsq(n):
                """x^2 tiles for chunk n (split scalar/vector engines)."""
                xn = xns[n]
                tiles = []
                for kb in range(KC):
                    xsq = stg.tile([128, 512], BF, tag="xsq", bufs=8,
                                   name="xsq")
                    nc.vector.tensor_mul(xsq, xn[:, kb, :], xn[:, kb, :])
                    tiles.append(xsq)
                xsqs[n] = tiles

            def emit_norm_reduce(n):
                """RMS scale for chunk n: xn *= rsqrt(mean(x^2)+eps)."""
                xn = xns[n]
                ss = uop.tile([128, 512], F32, tag="uo")
                for kb in range(KC):
                    nc.tensor.matmul(ss, onesf, xsqs[n][kb],
                                     start=(kb == 0), stop=(kb == KC - 1))
                lnm = stg.tile([1, 512], F32, tag="lnm")
                nc.scalar.activation(out=lnm, in_=ss[0:1, :], func=AF.Ln,
                                     bias=epst, scale=float(1.0 / C))
                r1 = stg.tile([1, 512], BF, tag="r1")
                nc.scalar.activation(out=r1, in_=lnm, func=AF.Exp, scale=-0.5)
                rbT = stg.tile([128, 512], BF, tag="rbT")
                nc.gpsimd.partition_broadcast(rbT, r1)
                for kb in range(KC):
                    nc.vector.tensor_mul(xn[:, kb, :], xn[:, kb, :], rbT)

            def rope_swap(src):
                """rot_half permutation via PE: qsw = J @ src (PSUM f32)."""
                qsw = scpp.tile([128, 512], F32, tag="scp")
                nc.tensor.matmul(qsw, swapm, src, start=True, stop=True)
                return qsw

            def rope_finish(src, qsw, dest_ap, nsl):
                """dest = src*cos + qsw*sinm (sign folded in sinmT)."""
                qsin = stg.tile([128, 512], BF, tag="qsin")
                nc.vector.tensor_mul(qsin, qsw, sinmT[:, nsl])
                qcos = stg.tile([128, 512], BF, tag="qcos")
                nc.vector.tensor_mul(qcos, src, cosT[:, nsl])
                nc.vector.tensor_add(dest_ap, qcos, qsin)

            def emit_kvq(n, h, qTs):
                """QKV projection + RoPE/v-transpose for head h, chunk n.

                PE-stream order: q-MMs, v-MMs, Jswap(q), k-MMs, vtrans(v) --
                each dependent PE op hides behind the next 16-matmul chain.
                The k rotate goes through a DMA partition swap (except chunk
                0, whose k feeds attention immediately)."""
                nsl = slice(n * 512, (n + 1) * 512)
                xn = xns[n]
                wmh = wms[(n, h)]

                def proj(sec, ptag="mm"):
                    if ptag == "mm":
                        ps = mmp.tile([128, 512], F32, tag="mm", name="ps")
                    elif ptag == "uo":
                        ps = uop.tile([128, 512], F32, tag="uo", name="ps3")
                    else:
                        ps = rsp.tile([128, 512], F32, tag="rs", name="ps2")
                    for kb in range(KC):
                        nc.tensor.matmul(ps, wmh[:, sec, kb, :],
                                         xn[:, kb, :],
                                         start=(kb == 0), stop=(kb == KC - 1))
                    qs = stg.tile([128, 512], BF, tag="qs", bufs=6,
                                  name="qs")
                    nc.scalar.activation(out=qs, in_=ps, func=AF.Identity,
                                         bias=bqkv[:, 3 * h + sec:
                                                   3 * h + sec + 1])
                    return qs

                qs_q = proj(2, ptag="rs" if h == 0 else "mm")
                qs_v = proj(1, ptag="uo" if h == 0 else "mm")
                qsw_q = rope_swap(qs_q)
                qs_k = proj(0)
                qT = stg.tile([128, 512], BF, tag=f"qT{h}")
                rope_finish(qs_q, qsw_q, qT, nsl)
                qTs[h] = qT
                vtp = scpp.tile([128, 4, 128], BF, tag="scp")
                for i2 in range(4):
                    nc.tensor.transpose(
                        vtp[:, i2, :], qs_v[:, i2 * 128:(i2 + 1) * 128],
                        ident)
                nc.vector.tensor_copy(vtrs[h][:, 4 * n:4 * n + 4, :], vtp)
                if n == 0:
                    qsw_k = rope_swap(qs_k)
                else:
                    qsw_k = stg.tile([128, 512], BF, tag="qswk", bufs=3)
                    nc.sync.dma_start(out=qsw_k[0:64, :],
                                      in_=qs_k[64:128, :])
                    nc.sync.dma_start(out=qsw_k[64:128, :],
                                      in_=qs_k[0:64, :])
                rope_finish(qs_k, qsw_k, kTs[h][:, nsl], nsl)

            def emit_attn_pair(n, pair, qTs, aTc):
                """Causal attention for two heads, j-loops interleaved."""
                nj = 4 * n + 4
                uo = {}
                rs = {}
                pts = {}
                usls = {}
                for h in pair:
                    uo[h] = uop.tile([128, 512], F32, tag="uo", name=f"uo{h}")
                    rs[h] = rsp.tile([128, 512], F32, tag="rs", name=f"rs{h}")

                def pv_rs(h, jb):
                    usl = usls[(h, jb)]
                    pt = pts[(h, jb)]
                    nc.tensor.matmul(uo[h][:, usl], vtrs[h][:, jb, :],
                                     pt[:, usl],
                                     start=(jb == 0), stop=(jb == nj - 1))
                    nc.tensor.matmul(rs[h][:, usl], onesf, pt[:, usl],
                                     start=(jb == 0), stop=(jb == nj - 1))

                for jb in range(nj):
                    r = jb - 4 * n
                    scps = {}
                    for h in pair:
                        jsl = slice(jb * 128, (jb + 1) * 128)
                        scp = scpp.tile([128, 512], F32, tag="scp")
                        if r >= 0:
                            u0 = 128 * r
                            usl = slice(u0, 512)
                            # causal mask pre-load: scp[:,u0:u0+128] = triT.T
                            nc.tensor.matmul(scp[:, u0:u0 + 128], triT,
                                             ident, start=True, stop=False)
                            nc.tensor.matmul(scp[:, usl], kTs[h][:, jsl],
                                             qTs[h][:, usl],
                                             start=False, stop=True)
                        else:
                            usl = slice(0, 512)
                            nc.tensor.matmul(scp, kTs[h][:, jsl], qTs[h],
                                             start=True, stop=True)
                        usls[(h, jb)] = usl
                        scps[h] = scp
                    if jb > 1:
                        for h in pair:
                            pv_rs(h, jb - 2)
                    for h in pair:
                        usl = usls[(h, jb)]
                        pt = stg.tile([128, 512], BF, tag="pt", bufs=8)
                        nc.scalar.activation(out=pt[:, usl],
                                             in_=scps[h][:, usl],
                                             func=AF.Exp, scale=SCALE)
                        pts[(h, jb)] = pt
                for jb2 in (nj - 2, nj - 1):
                    for h in pair:
                        pv_rs(h, jb2)
                for h in pair:
                    rcs = stg.tile([128, 512], F32, tag="rcs")
                    nc.vector.reciprocal_approx_fast(out=rcs, in_=rs[h])
                    nc.vector.tensor_mul(aTc[:, h, :], uo[h], rcs)

            def emit_oproj(n, aTc):
                for tb in range(4):
                    ost = stg.tile([128, C], BF, tag="ost", bufs=3)
                    for nn in range(4):
                        pso = mmp.tile([128, 512], F32, tag="mm")
                        for cb in range(HPC):
                            nc.tensor.matmul(
                                pso, aTc[:, cb, tb * 128:(tb + 1) * 128],
                                owT[:, cb, nn * 512:(nn + 1) * 512],
                                start=(cb == 0), stop=(cb == HPC - 1))
                        if nn % 2 == 0:
                            nc.scalar.activation(
                                out=ost[:, nn * 512:(nn + 1) * 512],
                                in_=pso, func=AF.Copy)
                        else:
                            nc.vector.tensor_copy(
                                ost[:, nn * 512:(nn + 1) * 512], pso)
                    nc.sync.dma_start(out=out_d[:, 4 * n + tb, :], in_=ost)

            # ---------------- main chunk pipeline --------------------------
            slots = [(n, h) for n in range(NT) for h in range(HPC)]

            def prefetch_w(idx):
                if idx >= len(slots) or slots[idx] in wms:
                    return
                n_, h_ = slots[idx]
                wms[(n_, h_)] = wp.tile([128, 3, KC, 128], BF,
                                        tag="wmh", name=f"wmh{n_}{h_}")
                nc.sync.dma_start(out=wms[(n_, h_)],
                                  in_=w_d[:, 3 * h_:3 * h_ + 3])

            prefetch_w(1)
            nc.sync.dma_start(out=owT, in_=ow_d[:, :, :])
            emit_xsq(0)
            emit_norm_reduce(0)
            for n in range(NT):
                if n < NT - 1:
                    xns[n + 1] = xkp.tile([128, KC, 512], BF, tag="xn",
                                          name=f"xn{n + 1}")
                    nc.sync.dma_start(out=xns[n + 1], in_=x_d[:, n + 1])
                qTs = {}
                aTc = stg.tile([128, HPC, 512], BF, tag="aTc")
                for h in (0, 1):
                    prefetch_w(4 * n + h + 2)
                    emit_kvq(n, h, qTs)
                emit_attn_pair(n, (0, 1), qTs, aTc)
                if n < NT - 1:
                    emit_xsq(n + 1)
                prefetch_w(4 * n + 4)
                emit_kvq(n, 2, qTs)
                if n < NT - 1:
                    emit_norm_reduce(n + 1)
                prefetch_w(4 * n + 5)
                emit_kvq(n, 3, qTs)
                emit_attn_pair(n, (2, 3), qTs, aTc)
                emit_oproj(n, aTc)

    nc.compile()
    return nc


def _get_nc():
    if "nc" not in _cache:
        _cache["nc"] = _build_nc()
    return _cache["nc"]


def _host_prep(x, rms_weight, qkv_w, qkv_b, o_w):
    """Build the per-core input maps (all bf16 except biases/masks)."""
    x = np.asarray(x, dtype=np.float32)
    rms_weight = np.asarray(rms_weight, dtype=np.float32)
    qkv_w = np.asarray(qkv_w, dtype=np.float32)
    qkv_b = np.asarray(qkv_b, dtype=np.float32)
    o_w = np.asarray(o_w, dtype=np.float32)

    w_eff = qkv_w * rms_weight[None, :]

    pos = np.arange(T, dtype=np.float32)
    inv_freq = (1.0 / (ROPE_BASE ** (np.arange(0, D, 2, dtype=np.float32)
                                     / D))).astype(np.float32)
    Fr = pos[None, :] * inv_freq[:, None]           # [64, T]
    cos_h = np.cos(Fr)
    sin_h = np.sin(Fr)
    cosT = np.concatenate([cos_h, cos_h], axis=0).astype(BF16NP)   # [128, T]
    sinmT = np.concatenate([-sin_h, sin_h], axis=0).astype(BF16NP)

    iu = np.arange(128)
    # mask[k, q] = 0 if q >= k else NEG ; triT = mask.T (lhsT for PE preload)
    mask = np.where(iu[None, :] >= iu[:, None], 0.0, NEG).astype(np.float32)
    triT = np.ascontiguousarray(mask.T).astype(BF16NP)
    identb = np.eye(128, dtype=np.float32).astype(BF16NP)
    swapm = np.zeros((128, 128), dtype=np.float32)
    swapm[np.arange(64), np.arange(64) + 64] = 1.0   # J[i, i+64] = 1
    swapm[np.arange(64) + 64, np.arange(64)] = 1.0   # J[i+64, i] = 1
    swapm = swapm.astype(BF16NP)
    onesb = np.ones((128, 128), dtype=np.float32).astype(BF16NP)

    # section order per head: k, v, q
    secoff = (C, 2 * C, 0)

    in_maps = []
    for core in range(NCORES):
        b = core // 4
        g = core % 4
        # xT packed [p, n, kb, ts]
        xT = np.ascontiguousarray(
            x[b].T.reshape(KC, 128, NT, 512).transpose(1, 2, 0, 3)
        ).astype(BF16NP)
        wblocks = []
        bcols = []
        for h in range(HPC):
            head = 4 * g + h
            for sec in range(3):
                r0 = secoff[sec] + head * 128
                sub = w_eff[r0:r0 + 128, :]                 # [128m, C]
                wblocks.append(sub.T.reshape(KC, 128, 128).transpose(1, 0, 2))
                bcols.append(qkv_b[r0:r0 + 128])
        wqkv = np.ascontiguousarray(
            np.stack(wblocks, axis=1)).astype(BF16NP)       # [128,12,KC,128]
        bq = np.ascontiguousarray(np.stack(bcols, axis=1)).astype(np.float32)
        owT = np.ascontiguousarray(
            o_w[:, g * 512:(g + 1) * 512].reshape(C, HPC, 128)
            .transpose(2, 1, 0)).astype(BF16NP)             # [128, HPC, C]
        in_maps.append({
            "xT": xT,
            "wqkv": wqkv,
            "bqkv": bq,
            "owT": owT,
            "cosT": cosT,
            "sinmT": sinmT,
            "triT": triT,
            "ident": identb,
            "swapm": swapm,
            "onesf": onesb,
        })
    return in_maps


def kernel(x, rms_weight, qkv_w, qkv_b, o_w, o_b):
    global last_exec_time_ns
    o_b = np.asarray(o_b, dtype=np.float32)
    in_maps = _host_prep(x, rms_weight, qkv_w, qkv_b, o_w)
    nc = _get_nc()

    trace = bool(int(os.environ.get("BASSK_TRACE", "0")))
    if trace:
        try:
            import ntff_shim
            ntff_shim.install()
        except Exception:
            pass
    res = None
    for attempt in range(4):
        try:
            res = run_bass_kernel_spmd(nc, in_maps, list(range(NCORES)),
                                       trace=trace)
            break
        except ImportError:
            # trace plumbing unavailable in this environment
            trace = False
        except Exception:
            if attempt == 3:
                raise
            import time
            time.sleep(5)
    last_exec_time_ns = res.exec_time_ns

    out = np.empty((B, T, C), dtype=np.float32)
    for b in range(B):
        acc = None
        for g in range(4):
            part = np.asarray(res.results[4 * b + g]["out"],
                              dtype=np.float32)
            part = part.transpose(1, 0, 2).reshape(T, C)
            acc = part if acc is None else acc + part
        out[b] = acc + o_b[None, :]
    return out
